# revision 1
# baseline (speedup 1.0000x reference)
"""Trainium2 Bass kernel for a 2-layer GCN over 2048 independent 25-node
KNN subgraphs (gnn_message_passing).

Strategy:
  - Each 25-node subgraph is independent -> the sparse scatter/gather
    aggregation is a dense per-graph 25x25 matmul. Host packs the
    normalized adjacency (transposed) into block-diagonal 125x125 tiles
    (5 graphs per tile) so the PE array contracts over 125 partitions.
  - Reassociate layer 1: relu(A @ (x @ W0)) == relu((A @ x) @ W0). With
    aggregation first, every matmul stays node-major and no on-chip
    transpose is ever needed (x is fed feature-major from the host).
  - Only the 5 center nodes per tile are needed downstream of the
    layer-2 aggregation, so (A @ h1) is computed for 5 targets only and
    the W1 transform runs once, weight-stationary, over all 260 centers.
  - Data parallel over 8 cores: 256 graphs (52 tiles, last one padded)
    per core; weights replicated.
"""

import os
import sys

import ml_dtypes
import numpy as np

for _p in ("/opt/trn_rl_repo", "/opt/trn_rl_repo/concourse"):
    if _p not in sys.path:
        sys.path.insert(0, _p)

import concourse.bass as bass
import concourse.tile as tile
from concourse import bacc, mybir
from concourse.bass_utils import run_bass_kernel_spmd

NCORES = 8
B = 2048            # graphs
K = 25              # nodes per graph
N = B * K           # 51200
GPC = B // NCORES   # 256 graphs per core
G = 5               # graphs packed per PE tile
P = G * K           # 125 partitions used per tile
NT = (GPC + G - 1) // G   # 52 tiles per core (last tile: 1 real graph)
SLOTS = NT * G      # 260 graph slots per core
NPAD = NT * P       # 6500 padded nodes per core
CP = 8              # padded center count (f32r needs even moving dims)
AW = 128            # at row width (125 block cols + 3 pad for alignment)
F0 = 128            # input features
F1 = 256            # hidden features

_f32 = mybir.dt.float32
_bf16 = mybir.dt.bfloat16

_compiled = {}


def _build_nc(mode):
    """Build + compile the per-core Bass program. mode selects the dtype
    of the SBUF-resident matmul operands:
      f32    - everything float32 (4 cy/col matmuls)
      f32r   - everything float32r (1 cy/col at >=256 moving cols)
      bf16   - everything bfloat16 (1 cy/col any width, fast ldweights)
      hybrid - f32r, but the tiny center-agg matmul (pure overhead) in bf16
    """
    mm_dt = {"f32": _f32, "f32r": mybir.dt.float32r, "bf16": _bf16,
             "hybrid": mybir.dt.float32r}[mode]
    c_dt = _bf16 if mode in ("bf16", "hybrid") else mm_dt
    nc = bacc.Bacc("TRN2", target_bir_lowering=False, debug=False,
                   num_devices=NCORES)

    # Inputs declared with the matmul dtype (float32r is bit-identical to
    # f32; np mapping stays float32) so plain DMAs are not dtype casts.
    xT_d = nc.dram_tensor("xT", [F0, NPAD], mm_dt, kind="ExternalInput")
    # partition-major so the whole tensor DMAs as 125 contiguous 27KB rows
    at_d = nc.dram_tensor("at", [P, NT, AW], mm_dt, kind="ExternalInput")
    w0_d = nc.dram_tensor("w0", [F0, F1], mm_dt, kind="ExternalInput")
    w1_d = nc.dram_tensor("w1", [F1, F1], mm_dt, kind="ExternalInput")
    wl_d = nc.dram_tensor("wl", [128, 2], mm_dt, kind="ExternalInput")
    atc_d = nc.dram_tensor("atc", [P, NT, CP], c_dt, kind="ExternalInput")
    out_d = nc.dram_tensor("out", [1, SLOTS], _f32, kind="ExternalOutput")

    relu = mybir.ActivationFunctionType.Relu

    with tile.TileContext(nc) as tc:
        with (
            tc.tile_pool(name="const", bufs=1) as cpool,
            tc.tile_pool(name="qp", bufs=3) as qp,
            tc.tile_pool(name="h1p", bufs=3) as h1p,
            tc.tile_pool(name="outp", bufs=1) as outp,
            tc.tile_pool(name="ps_q", bufs=2, space=bass.MemorySpace.PSUM) as ps_q,
            tc.tile_pool(name="ps_h1", bufs=2, space=bass.MemorySpace.PSUM) as ps_h1,
            tc.tile_pool(name="ps_p2", bufs=2, space=bass.MemorySpace.PSUM) as ps_p2,
            tc.tile_pool(name="ps_f", bufs=1, space=bass.MemorySpace.PSUM) as ps_f,
        ):
            # ---- resident constants (weights first: tile 0 needs them) ----
            w0 = cpool.tile([F0, F1], mm_dt)
            nc.sync.dma_start(w0[:], w0_d[:])
            w1 = cpool.tile([128, 2 * F1], mm_dt)   # [fic packed] x [fo]
            nc.sync.dma_start(w1[:, 0:F1], w1_d[0:128, :])
            nc.sync.dma_start(w1[:, F1:2 * F1], w1_d[128:256, :])
            wl = cpool.tile([128, 2], mm_dt)
            nc.sync.dma_start(wl[:], wl_d[:])
            xT = cpool.tile([F0, NPAD], mm_dt)
            # layer-2 agg (centers), fT-major, chunk-major so the W1-phase
            # moving operand p2a[:, fi, :] is a contiguous [128, 260]
            p2a = cpool.tile([128, 2, SLOTS], mm_dt)

            at_all = cpool.tile([P, NT, AW], mm_dt)
            atc_all = cpool.tile([P, NT, CP], c_dt)
            nc.sync.dma_start(atc_all[:], atc_d[:])
            nchunk = 8
            cw = NPAD // nchunk
            aw = NT // nchunk   # 6.5 -> use ceil split below
            bounds = [round(NT * c / nchunk) for c in range(nchunk + 1)]
            for c in range(nchunk):
                nc.sync.dma_start(xT[:, c * cw:(c + 1) * cw],
                                  xT_d[:, c * cw:(c + 1) * cw])
                lo, hi = bounds[c], bounds[c + 1]
                nc.sync.dma_start(at_all[:, lo:hi, :], at_d[:, lo:hi, :])

            # ---- per-tile loop ----
            for i in range(NT):
                at_t = at_all[:, i, :]

                # q = x @ W0   (node-major out [125, 256])
                q_ps = ps_q.tile([P, F1], _f32)
                nc.tensor.matmul(q_ps[:], xT[:, i * P:(i + 1) * P], w0[:],
                                 start=True, stop=True)
                q_sb = qp.tile([P, F1], mm_dt)
                nc.vector.tensor_copy(q_sb[:], q_ps[:])

                # h1 = relu(AT.T @ q) = relu(A @ x @ W0)
                h1_ps = ps_h1.tile([P, F1], _f32)
                nc.tensor.matmul(h1_ps[:], at_t[:, 0:P], q_sb[:],
                                 start=True, stop=True)
                h1_sb = h1p.tile([P, F1], c_dt)
                nc.scalar.activation(h1_sb[:], h1_ps[:], relu)

                # p2cT[f, t] = sum_s h1[s, f] * ATc[s, t] (5 centers, padded
                # to 8 cols: f32r rejects small/odd moving free-dims)
                p2_ps = ps_p2.tile([128, 2, CP], _f32)
                for c in range(2):
                    nc.tensor.matmul(p2_ps[:, c, :],
                                     h1_sb[:, c * 128:(c + 1) * 128],
                                     atc_all[:, i, :],
                                     start=True, stop=True)
                nc.vector.tensor_copy(p2a[:, :, i * G:(i + 1) * G],
                                      p2_ps[:, :, 0:G])

            # ---- W1 transform over all centers (weight stationary) ----
            h3_sb = cpool.tile([128, 2, SLOTS], mm_dt)
            for fo in range(2):
                h3_ps = ps_f.tile([128, SLOTS], _f32)
                for fi in range(2):
                    nc.tensor.matmul(h3_ps[:],
                                     w1[:, fi * F1 + fo * 128:fi * F1 + fo * 128 + 128],
                                     p2a[:, fi, :],
                                     start=(fi == 0), stop=(fi == 1))
                nc.scalar.activation(h3_sb[:, fo, :], h3_ps[:], relu)

            # ---- out = relu(h3).T @ Wlin ----
            out_ps = ps_f.tile([1, SLOTS], _f32)
            for fo in range(2):
                nc.tensor.matmul(out_ps[:], wl[:, fo:fo + 1], h3_sb[:, fo, :],
                                 start=(fo == 0), stop=(fo == 1))
            out_sb = outp.tile([1, SLOTS], _f32)
            nc.vector.tensor_copy(out_sb[:], out_ps[:])
            nc.sync.dma_start(out_d[:], out_sb[:])

    nc.compile()
    return nc


def _get_nc(mode):
    if mode not in _compiled:
        _compiled[mode] = _build_nc(mode)
    return _compiled[mode]


def _host_prep(mode, x, edge_weight, W0, W1, Wlin, edge_index):
    mm_np = ml_dtypes.bfloat16 if mode == "bf16" else np.float32
    c_np = ml_dtypes.bfloat16 if mode in ("bf16", "hybrid") else np.float32
    src = edge_index[0].astype(np.int64)
    tgt = edge_index[1].astype(np.int64)
    b = src // K
    sl = src - b * K
    tl = tgt - (tgt // K) * K

    # dense raw adjacency per graph, indexed [b, t, s]
    idx = (b * K + tl) * K + sl
    Araw = np.bincount(idx, weights=edge_weight.astype(np.float64),
                       minlength=B * K * K).astype(np.float32).reshape(B, K, K)
    deg = Araw.sum(axis=2)                      # weighted in-degree [B, K]
    with np.errstate(divide="ignore"):
        dinv = np.where(deg > 0, 1.0 / np.sqrt(deg), 0.0).astype(np.float32)
    An = Araw * dinv[:, :, None] * dinv[:, None, :]   # [b, t, s]
    ATn = np.ascontiguousarray(An.transpose(0, 2, 1))  # [b, s, t]

    # scatter graphs into per-core padded slots
    ATs = np.zeros((NCORES, SLOTS, K, K), np.float32)
    ATs[:, :GPC] = ATn.reshape(NCORES, GPC, K, K)
    ATs = ATs.reshape(NCORES, NT, G, K, K)

    at = np.zeros((NCORES, NT, P, AW), np.float32)
    bd = at[..., :P].reshape(NCORES, NT, G, K, G, K)
    atc = np.zeros((NCORES, NT, P, CP), np.float32)
    cent = atc[..., :G].reshape(NCORES, NT, G, K, G)
    for g in range(G):
        bd[:, :, g, :, g, :] = ATs[:, :, g]          # block-diagonal AT
        cent[:, :, g, :, g] = ATs[:, :, g, :, 0]     # center (t_local=0) col
    # partition-major device layout
    at = np.ascontiguousarray(at.transpose(0, 2, 1, 3).astype(mm_np))
    atc = np.ascontiguousarray(atc.transpose(0, 2, 1, 3).astype(c_np))

    xp = np.zeros((NCORES, NPAD, F0), np.float32)
    xp[:, :GPC * K] = x.reshape(NCORES, GPC * K, F0)
    xT = np.ascontiguousarray(xp.transpose(0, 2, 1).astype(mm_np))

    wl = np.ascontiguousarray(Wlin.reshape(2, 128).T.astype(mm_np))

    in_maps = []
    for c in range(NCORES):
        in_maps.append({
            "xT": xT[c],
            "at": np.ascontiguousarray(at[c]),
            "atc": np.ascontiguousarray(atc[c]),
            "w0": np.ascontiguousarray(W0.astype(mm_np)),
            "w1": np.ascontiguousarray(W1.astype(mm_np)),
            "wl": wl,
        })
    return in_maps


def _run(inputs, mode="f32r", trace=False):
    nc = _get_nc(mode)
    in_maps = _host_prep(mode, **inputs)
    res = run_bass_kernel_spmd(nc, in_maps, core_ids=list(range(NCORES)),
                               trace=trace)
    out = np.empty((B, 1), np.float32)
    for c in range(NCORES):
        out[c * GPC:(c + 1) * GPC, 0] = res.results[c]["out"][0, :GPC]
    return out, res


def kernel(**inputs):
    mode = os.environ.get("GCN_DTYPE", "f32r")
    out, _ = _run(inputs, mode=mode, trace=False)
    return out



# revision 3
# speedup vs baseline: 1.1027x; 1.1027x over previous
"""Trainium2 Bass kernel for a 2-layer GCN over 2048 independent 25-node
KNN subgraphs (gnn_message_passing).

Strategy (mode "opt"):
  - Each 25-node subgraph is independent -> the sparse scatter/gather
    aggregation is a dense per-graph 25x25 matmul. Host packs the
    normalized adjacency (transposed) into block-diagonal 128x128 tiles
    (5 graphs per tile, 3 pad rows/cols) so every stationary operand is
    exactly 128 columns wide -> the compiler's Fast Weight Load kicks in
    (requires non-fp32 dtype and a 128-col weight tile).
  - All matmul operands are bf16 (1 cy/col at any width); accumulation
    stays f32 in PSUM. rel-err budget is 2e-2, bf16 lands ~8e-3.
  - Per super-tile (2 PE tiles): q = x@W0 (x-tile stationary, w0
    moving), h1 = relu(AT.T @ q) (at-tile stationary), then the
    layer-2 center aggregation writes straight into a PSUM accumulator
    (one 5-col slice per tile) so no per-tile copy is needed.
  - relu/copy PSUM drains are split across scalar+vector engines so no
    single drain engine limits the tensor pipe.
  - DMA: inputs are packed into 3 dram tensors (w0+xT / at+atc / w1+wl)
    and fetched with a few large, prioritized transfers split across
    both HWDGE rings (sync + scalar) so descriptor generation (~1.3us
    per 128-partition DMA) does not serialize the startup.
  - Data parallel over 8 cores: 256 graphs (52 tiles) per core.
"""

import os
import sys

import ml_dtypes
import numpy as np

for _p in ("/opt/trn_rl_repo", "/opt/trn_rl_repo/concourse"):
    if _p not in sys.path:
        sys.path.insert(0, _p)

import concourse.bass as bass
import concourse.tile as tile
from concourse import bacc, mybir
from concourse.bass_utils import run_bass_kernel_spmd

NCORES = 8
B = 2048            # graphs
K = 25              # nodes per graph
N = B * K           # 51200
GPC = B // NCORES   # 256 graphs per core
G = 5               # graphs packed per PE tile
P = G * K           # 125 real partitions used per tile
PT = 128            # padded tile width (FWL needs 128-col stationaries)
NT = (GPC + G - 1) // G   # 52 tiles per core (last tile: 1 real graph)
SLOTS = NT * G      # 260 graph slots per core
NPAD = NT * PT      # 6656 padded node slots per core
CP = 5              # center-extraction columns per tile
AW = PT + CP        # 133 cols per tile in the packed at+atc tensor
F0 = 128            # input features
F1 = 256            # hidden features
XW = F1 + NPAD      # w0 (256 cols) + xT, packed in one dram tensor

_f32 = mybir.dt.float32
_bf16 = mybir.dt.bfloat16

_compiled = {}


def _build_nc_opt():
    nc = bacc.Bacc("TRN2", target_bir_lowering=False, debug=False,
                   num_devices=NCORES)

    xw_d = nc.dram_tensor("xw", [F0, XW], _bf16, kind="ExternalInput")
    ac_d = nc.dram_tensor("ac", [PT, NT, AW], _bf16, kind="ExternalInput")
    wsm_d = nc.dram_tensor("wsm", [128, 2 * F1 + 2], _bf16,
                           kind="ExternalInput")
    out_d = nc.dram_tensor("out", [1, SLOTS], _f32, kind="ExternalOutput")

    relu = mybir.ActivationFunctionType.Relu
    # chunk boundaries (in tiles) for the pipelined input DMAs
    xb = [0, 10, 20, 36, NT]

    with tile.TileContext(nc) as tc:
        with (
            tc.tile_pool(name="const", bufs=1) as cpool,
            tc.tile_pool(name="qp", bufs=3) as qp,
            tc.tile_pool(name="h1p", bufs=3) as h1p,
            tc.tile_pool(name="outp", bufs=1) as outp,
            tc.tile_pool(name="ps_q", bufs=2, space=bass.MemorySpace.PSUM) as ps_q,
            tc.tile_pool(name="ps_h1", bufs=2, space=bass.MemorySpace.PSUM) as ps_h1,
            tc.tile_pool(name="ps_p2a", bufs=1, space=bass.MemorySpace.PSUM) as ps_p2a,
            tc.tile_pool(name="ps_p2b", bufs=1, space=bass.MemorySpace.PSUM) as ps_p2b,
            tc.tile_pool(name="ps_f", bufs=1, space=bass.MemorySpace.PSUM) as ps_f,
        ):
            xw = cpool.tile([F0, XW], _bf16)
            ac = cpool.tile([PT, NT, AW], _bf16)
            wsm = cpool.tile([128, 2 * F1 + 2], _bf16)

            # prioritized input DMAs: first chunks first, split across the
            # two HWDGE rings (sync: x-side, scalar: adjacency-side)
            for c in range(4):
                lo, hi = xb[c], xb[c + 1]
                xlo = 0 if c == 0 else F1 + lo * PT
                nc.sync.dma_start(xw[:, xlo:F1 + hi * PT],
                                  xw_d[:, xlo:F1 + hi * PT])
                nc.scalar.dma_start(ac[:, lo:hi, :], ac_d[:, lo:hi, :])
            nc.sync.dma_start(wsm[:], wsm_d[:])

            w0 = xw[:, 0:F1]
            p2acc_a = ps_p2a.tile([128, SLOTS], _f32)
            p2acc_b = ps_p2b.tile([128, SLOTS], _f32)

            for st in range(NT // 2):
                a, b = 2 * st, 2 * st + 1

                q_ps = ps_q.tile([128, 2 * F1], _f32)
                nc.tensor.matmul(q_ps[:, 0:F1],
                                 xw[:, F1 + a * PT:F1 + (a + 1) * PT], w0,
                                 start=True, stop=True)
                nc.tensor.matmul(q_ps[:, F1:2 * F1],
                                 xw[:, F1 + b * PT:F1 + (b + 1) * PT], w0,
                                 start=True, stop=True)
                q_sb = qp.tile([128, 2 * F1], _bf16)
                nc.vector.tensor_copy(q_sb[:], q_ps[:])

                h1_ps = ps_h1.tile([128, 2 * F1], _f32)
                nc.tensor.matmul(h1_ps[:, 0:F1], ac[:, a, 0:PT],
                                 q_sb[:, 0:F1], start=True, stop=True)
                nc.tensor.matmul(h1_ps[:, F1:2 * F1], ac[:, b, 0:PT],
                                 q_sb[:, F1:2 * F1], start=True, stop=True)
                h1_sb = h1p.tile([128, 2 * F1], _bf16)
                nc.scalar.activation(h1_sb[:, 0:F1], h1_ps[:, 0:F1], relu)
                nc.vector.tensor_scalar_max(h1_sb[:, F1:2 * F1],
                                            h1_ps[:, F1:2 * F1], 0.0)

                for t, off in ((a, 0), (b, 2 * 128)):
                    atc_t = ac[:, t, PT:PT + CP]
                    nc.tensor.matmul(p2acc_a[:, t * G:(t + 1) * G],
                                     h1_sb[:, off:off + 128], atc_t,
                                     start=True, stop=True)
                    nc.tensor.matmul(p2acc_b[:, t * G:(t + 1) * G],
                                     h1_sb[:, off + 128:off + 256], atc_t,
                                     start=True, stop=True)

            # ---- W1 transform over all centers (weight stationary) ----
            p2s = cpool.tile([128, 2, SLOTS], _bf16)
            nc.vector.tensor_copy(p2s[:, 0, :], p2acc_a[:])
            nc.vector.tensor_copy(p2s[:, 1, :], p2acc_b[:])

            h3_sb = cpool.tile([128, 2, SLOTS], _bf16)
            for fo in range(2):
                h3_ps = ps_f.tile([128, SLOTS], _f32)
                for fi in range(2):
                    nc.tensor.matmul(h3_ps[:],
                                     wsm[:, fi * F1 + fo * 128:fi * F1 + fo * 128 + 128],
                                     p2s[:, fi, :],
                                     start=(fi == 0), stop=(fi == 1))
                nc.scalar.activation(h3_sb[:, fo, :], h3_ps[:], relu)

            # ---- out = relu(h3).T @ Wlin ----
            out_ps = ps_f.tile([1, SLOTS], _f32)
            for fo in range(2):
                nc.tensor.matmul(out_ps[:], wsm[:, 2 * F1 + fo:2 * F1 + fo + 1],
                                 h3_sb[:, fo, :],
                                 start=(fo == 0), stop=(fo == 1))
            out_sb = outp.tile([1, SLOTS], _f32)
            nc.vector.tensor_copy(out_sb[:], out_ps[:])
            nc.sync.dma_start(out_d[:], out_sb[:])

    nc.compile()
    return nc


def _get_nc(mode):
    if mode not in _compiled:
        _compiled[mode] = _build_nc_opt()
    return _compiled[mode]


def _host_prep_opt(x, edge_weight, W0, W1, Wlin, edge_index):
    bf = ml_dtypes.bfloat16
    src = edge_index[0].astype(np.int64)
    tgt = edge_index[1].astype(np.int64)
    b = src // K
    sl = src - b * K
    tl = tgt - (tgt // K) * K

    # dense raw adjacency per graph, indexed [b, t, s]
    idx = (b * K + tl) * K + sl
    Araw = np.bincount(idx, weights=edge_weight.astype(np.float64),
                       minlength=B * K * K).astype(np.float32).reshape(B, K, K)
    deg = Araw.sum(axis=2)                      # weighted in-degree [B, K]
    with np.errstate(divide="ignore"):
        dinv = np.where(deg > 0, 1.0 / np.sqrt(deg), 0.0).astype(np.float32)
    An = Araw * dinv[:, :, None] * dinv[:, None, :]   # [b, t, s]
    ATn = np.ascontiguousarray(An.transpose(0, 2, 1))  # [b, s, t]

    # scatter graphs into per-core padded slots: [NC, NT, G, K(sl), K(tl)]
    ATs = np.zeros((NCORES, SLOTS, K, K), np.float32)
    ATs[:, :GPC] = ATn.reshape(NCORES, GPC, K, K)
    ATs = ATs.reshape(NCORES, NT, G, K, K)

    # packed adjacency tensor: [NC, PT(s), NT, PT(t) block-diag | CP centers]
    ac = np.zeros((NCORES, PT, NT, AW), np.float32)
    bd = ac[:, :P, :, :P].reshape(NCORES, G, K, NT, G, K)
    cent = ac[:, :P, :, PT:].reshape(NCORES, G, K, NT, G)
    for g in range(G):
        bd[:, g, :, :, g, :] = ATs[:, :, g].transpose(0, 2, 1, 3)
        cent[:, g, :, :, g] = ATs[:, :, g, :, 0].transpose(0, 2, 1)
    ac = ac.astype(bf)

    # packed x tensor: [NC, F0, 256 (w0) + NT*PT (xT, tile-padded)]
    xpad = np.zeros((NCORES, NT * P, F0), np.float32)
    xpad[:, :GPC * K] = x.reshape(NCORES, GPC * K, F0)
    xw = np.zeros((NCORES, F0, XW), np.float32)
    xw[:, :, 0:F1] = W0[None, :, :]
    xw[:, :, F1:].reshape(NCORES, F0, NT, PT)[:, :, :, :P] = \
        xpad.reshape(NCORES, NT, P, F0).transpose(0, 3, 1, 2)
    xw = xw.astype(bf)

    # packed small weights: [128, w1(fi0)x256 | w1(fi1)x256 | wl x2]
    wsm = np.zeros((128, 2 * F1 + 2), np.float32)
    wsm[:, 0:F1] = W1[0:128, :]
    wsm[:, F1:2 * F1] = W1[128:256, :]
    wsm[:, 2 * F1:] = Wlin.reshape(2, 128).T
    wsm = np.ascontiguousarray(wsm.astype(bf))

    in_maps = []
    for c in range(NCORES):
        in_maps.append({
            "xw": np.ascontiguousarray(xw[c]),
            "ac": np.ascontiguousarray(ac[c]),
            "wsm": wsm,
        })
    return in_maps


def _run(inputs, mode="opt", trace=False):
    nc = _get_nc(mode)
    in_maps = _host_prep_opt(**inputs)
    res = run_bass_kernel_spmd(nc, in_maps, core_ids=list(range(NCORES)),
                               trace=trace)
    out = np.empty((B, 1), np.float32)
    for c in range(NCORES):
        out[c * GPC:(c + 1) * GPC, 0] = res.results[c]["out"][0, :GPC]
    return out, res


def kernel(**inputs):
    out, _ = _run(inputs, mode="opt", trace=False)
    return out
